# revision 7
# baseline (speedup 1.0000x reference)
"""Trainium2 Bass kernel for nn_DivergenceRN (gnn_message_passing).

Reference computes, per batch b:
    Z_XX[b,i,:] = max_j relu(X[b,j]@W1a_xx + X[b,i]@W1c_xx + b1_xx) @ W_xx2
    Z_YX[b,i,:] = max_j relu(Y[b,j]@W1a_yx + X[b,i]@W1c_yx + b1_yx) @ W_yx2
    Z = sum_i (Z_XX - Z_YX);  out = relu(cat(Z,Z)@Wd1+bd1)@Wd2+bd2
(The YY / XY branches in the reference are dead code — output-independent.)

Device layout: partitions = 64 h-channels x {xx, yx} = 128; free dim = j.
Per (b,i): relu_pre = relu(pa_stacked + pc_col)  (ACT / GPSIMD / DVE),
one float32r matmul vs blockdiag(W_xx2, W_yx2)  (PE), max over j (DVE
reduce, batched over 4 PSUM banks), sum over i into a strip buffer.
Sharding: i in [0,384) split across 8 cores (48 rows per core per batch).
Host does the tiny decoder + bias folding.
"""

import numpy as np

import concourse.bacc as bacc
import concourse.bass as bass
import concourse.mybir as mybir
import concourse.tile as tile
from concourse.bass_utils import run_bass_kernel_spmd

B, N, M, D, H = 4, 384, 384, 64, 64
NCORES = 8
NI = N // NCORES          # i-rows per core per batch
G = 4                     # i's reduced per DVE tensor_reduce (PSUM banks)
P = 2 * H                 # 128 partitions: h x {xx, yx}
BLOB_W = B * N + B * NI + 3 * P + 1   # packed input blob columns

F32 = mybir.dt.float32
F32R = mybir.dt.float32r
AX = mybir.AxisListType
ALU = mybir.AluOpType
ACTF = mybir.ActivationFunctionType

# relu_pre engine assignment, cycled over the flat (b, i) index.
PRE_PATTERN = ("act", "gpsimd")


def build_nc():
    nc = bacc.Bacc("TRN2", target_bir_lowering=False)

    # All inputs packed into one [128, BLOB_W] f32 blob: a single DMA =
    # a single semaphore (per-instruction sync-wait limits are tiny).
    blob = nc.dram_tensor("blob", [P, BLOB_W], F32R, kind="ExternalInput")
    out = nc.dram_tensor("out", [P, B], F32, kind="ExternalOutput")

    with tile.TileContext(nc) as tc:
        with (
            tc.tile_pool(name="singles", bufs=1) as singles,
            tc.tile_pool(name="rp", bufs=8) as rp_pool,
            tc.tile_pool(name="psum", bufs=2, space="PSUM") as psum_pool,
        ):
            blob_s = singles.tile([P, BLOB_W], F32R)
            pa_sb = singles.tile([P, B, N], F32)
            pc_sb = singles.tile([P, B, NI], F32)
            strip = singles.tile([P, B, NI], F32)
            acc = singles.tile([P, B], F32)

            nc.sync.dma_start(out=blob_s[:, :], in_=blob[:, :])
            o = 0
            xyt_s = blob_s[:, o : o + B * N].rearrange("p (b n) -> p b n", b=B)
            o += B * N
            xit_s = blob_s[0:D, o : o + B * NI].rearrange(
                "p (b i) -> p b i", b=B
            )
            o += B * NI
            w1ad_s = blob_s[:, o : o + P]
            o += P
            w1c_s = blob_s[0:D, o : o + P]
            o += P
            w2bd_s = blob_s[:, o : o + P]
            o += P
            b1_s = blob_s[:, o : o + 1].bitcast(F32)
            o += 1
            assert o == BLOB_W

            # Setup per b: pa_stacked = blockdiag(W1a) @ [X^T; Y^T],
            # pc_stacked = [W1c_xx | W1c_yx]^T-style cat @ Xi^T, + b1.
            for b in range(B):
                ps = psum_pool.tile([P, G, 512], F32, tag="mm")
                nc.tensor.matmul(
                    ps[:, 0, 0:N],
                    lhsT=w1ad_s,
                    rhs=xyt_s[:, b, :],
                    start=True,
                    stop=True,
                )
                nc.vector.tensor_copy(pa_sb[:, b, :], ps[:, 0, 0:N])
                nc.tensor.matmul(
                    ps[:, 1, 0:NI],
                    lhsT=w1c_s,
                    rhs=xit_s[:, b, :],
                    start=True,
                    stop=True,
                )
                nc.vector.tensor_scalar_add(
                    out=pc_sb[:, b, :],
                    in0=ps[:, 1, 0:NI],
                    scalar1=b1_s,
                )

            # Main loop.
            flat = 0
            for b in range(B):
                for ig in range(NI // G):
                    ps = psum_pool.tile([P, G, 512], F32, tag="mm")
                    for g in range(G):
                        i = ig * G + g
                        rp = rp_pool.tile([P, N], F32R)
                        eng = PRE_PATTERN[flat % len(PRE_PATTERN)]
                        flat += 1
                        if eng == "act":
                            nc.scalar.activation(
                                out=rp[:, :],
                                in_=pa_sb[:, b, :],
                                func=ACTF.Relu,
                                bias=pc_sb[:, b, i : i + 1],
                                scale=1.0,
                            )
                        elif eng == "gpsimd":
                            nc.gpsimd.tensor_scalar(
                                out=rp[:, :],
                                in0=pa_sb[:, b, :],
                                scalar1=pc_sb[:, b, i : i + 1],
                                scalar2=0.0,
                                op0=ALU.add,
                                op1=ALU.max,
                            )
                        else:
                            nc.vector.tensor_scalar(
                                out=rp[:, :],
                                in0=pa_sb[:, b, :],
                                scalar1=pc_sb[:, b, i : i + 1],
                                scalar2=0.0,
                                op0=ALU.add,
                                op1=ALU.max,
                            )
                        nc.tensor.matmul(
                            ps[:, g, 0:N],
                            lhsT=w2bd_s,
                            rhs=rp[:, :],
                            start=True,
                            stop=True,
                        )
                    nc.vector.tensor_reduce(
                        out=strip[:, b, ig * G : ig * G + G],
                        in_=ps[:, :, 0:N],
                        axis=AX.X,
                        op=ALU.max,
                    )

            nc.vector.tensor_reduce(
                out=acc[:, :], in_=strip[:, :, :], axis=AX.X, op=ALU.add
            )
            nc.sync.dma_start(out=out[:, :], in_=acc[:, :])

    nc.compile()
    return nc


def _prep_inputs(X, Y, W_xx1, W_yx1, b_xx1, b_yx1, W_xx2, W_yx2):
    """Host-side input prep shared by all cores (except xit)."""
    f = np.float32
    XYT = np.ascontiguousarray(
        np.concatenate([X.transpose(0, 2, 1), Y.transpose(0, 2, 1)], axis=1), f
    )  # [B, 128, N]
    W1ad = np.zeros((P, P), f)
    W1ad[:D, :H] = W_xx1[:D]
    W1ad[D:, H:] = W_yx1[:D]
    W1c = np.ascontiguousarray(np.concatenate([W_xx1[D:], W_yx1[D:]], axis=1), f)
    b1v = np.concatenate([b_xx1, b_yx1]).reshape(P, 1).astype(f)
    W2bd = np.zeros((P, P), f)
    W2bd[:H, :H] = W_xx2
    W2bd[H:, H:] = W_yx2
    return XYT, W1ad, W1c, b1v, W2bd


def _pack_blob(XYT, XiT, W1ad, W1c, b1v, W2bd):
    """Pack all per-core inputs into the [P, BLOB_W] blob (see build_nc)."""
    f = np.float32
    blob = np.zeros((P, BLOB_W), f)
    o = 0
    blob[:, o : o + B * N] = XYT.transpose(1, 0, 2).reshape(P, B * N)
    o += B * N
    blob[:D, o : o + B * NI] = XiT.transpose(1, 0, 2).reshape(D, B * NI)
    o += B * NI
    blob[:, o : o + P] = W1ad
    o += P
    blob[:D, o : o + P] = W1c
    o += P
    blob[:, o : o + P] = W2bd
    o += P
    blob[:, o : o + 1] = b1v
    o += 1
    assert o == BLOB_W
    return blob


def kernel(
    X, Y,
    W_xx1, b_xx1, W_xx2, b_xx2,
    W_xy1, b_xy1, W_xy2, b_xy2,
    W_yx1, b_yx1, W_yx2, b_yx2,
    W_yy1, b_yy1, W_yy2, b_yy2,
    Wd1, bd1, Wd2, bd2,
    _trace=False, _tmpdir=None,
):
    f = np.float32
    X = np.asarray(X, f)
    Y = np.asarray(Y, f)
    XYT, W1ad, W1c, b1v, W2bd = _prep_inputs(
        X, Y, W_xx1, W_yx1, b_xx1, b_yx1, W_xx2, W_yx2
    )

    in_maps = []
    for c in range(NCORES):
        XiT = np.ascontiguousarray(
            X[:, c * NI : (c + 1) * NI, :].transpose(0, 2, 1), f
        )  # [B, 64, NI]
        in_maps.append({"blob": _pack_blob(XYT, XiT, W1ad, W1c, b1v, W2bd)})

    nc = build_nc()
    res = run_bass_kernel_spmd(
        nc,
        in_maps,
        core_ids=list(range(NCORES)),
        trace=_trace,
        tmpdir=_tmpdir,
    )
    acc = np.zeros((P, B), np.float64)
    for r in res.results:
        acc += r["out"].astype(np.float64)
    acc = acc.astype(f)

    # acc[k, b] = sum_i max_j (relu_pre @ W2)[k]  for xx (k<64) / yx (k>=64)
    Zdiff = (acc[:H] - acc[H:]).T + N * (b_xx2 - b_yx2)[None, :]  # [B, H]
    z = np.concatenate([Zdiff, Zdiff], axis=1).astype(f)  # [B, 2H]
    h = np.maximum(z @ Wd1 + bd1, 0.0).astype(f)
    outv = (h @ Wd2 + bd2).astype(f)
    if _trace:
        return outv, res
    return outv


# revision 8
# speedup vs baseline: 3.7390x; 3.7390x over previous
"""Trainium2 Bass kernel for nn_DivergenceRN (gnn_message_passing).

Reference computes, per batch b:
    Z_XX[b,i,:] = max_j relu(X[b,j]@W1a_xx + X[b,i]@W1c_xx + b1_xx) @ W_xx2
    Z_YX[b,i,:] = max_j relu(Y[b,j]@W1a_yx + X[b,i]@W1c_yx + b1_yx) @ W_yx2
    Z = sum_i (Z_XX - Z_YX);  out = relu(cat(Z,Z)@Wd1+bd1)@Wd2+bd2
(The YY / XY branches in the reference are dead code — output-independent.)

Device layout: partitions = 64 h-channels x {xx, yx} = 128; free dim = j.
Per (b,i): relu_pre = relu(pa_stacked + pc_col)  (ACT / GPSIMD / DVE),
one float32r matmul vs blockdiag(W_xx2, W_yx2)  (PE), max over j (DVE
reduce, batched over 4 PSUM banks), sum over i into a strip buffer.
Sharding: i in [0,384) split across 8 cores (48 rows per core per batch).
Host does the tiny decoder + bias folding.
"""

import numpy as np

import concourse.bacc as bacc
import concourse.bass as bass
import concourse.mybir as mybir
import concourse.tile as tile
from concourse.bass_utils import run_bass_kernel_spmd

B, N, M, D, H = 4, 384, 384, 64, 64
NCORES = 8
NI = N // NCORES          # i-rows per core per batch
G = 4                     # i's reduced per DVE tensor_reduce (PSUM banks)
P = 2 * H                 # 128 partitions: h x {xx, yx}
BLOB_W = B * N + B * NI + 3 * P + 1   # packed input blob columns

F32 = mybir.dt.float32
F32R = mybir.dt.float32r
AX = mybir.AxisListType
ALU = mybir.AluOpType
ACTF = mybir.ActivationFunctionType

# relu_pre engine assignment, cycled over the flat (b, i) index.
PRE_PATTERN = ("act", "act", "act", "act", "dve")


def build_nc():
    nc = bacc.Bacc("TRN2", target_bir_lowering=False)

    # All inputs packed into one [128, BLOB_W] f32 blob: a single DMA =
    # a single semaphore (per-instruction sync-wait limits are tiny).
    blob = nc.dram_tensor("blob", [P, BLOB_W], F32R, kind="ExternalInput")
    out = nc.dram_tensor("out", [P, B], F32, kind="ExternalOutput")

    with tile.TileContext(nc) as tc:
        with (
            tc.tile_pool(name="singles", bufs=1) as singles,
            tc.tile_pool(name="rp", bufs=12) as rp_pool,
            tc.tile_pool(name="psum", bufs=2, space="PSUM") as psum_pool,
        ):
            blob_s = singles.tile([P, BLOB_W], F32R)
            pa_sb = singles.tile([P, B, N], F32)
            pc_sb = singles.tile([P, B, NI], F32)
            strip = singles.tile([P, B, NI], F32)
            acc = singles.tile([P, B], F32)

            nc.sync.dma_start(out=blob_s[:, :], in_=blob[:, :])
            o = 0
            xyt_s = blob_s[:, o : o + B * N].rearrange("p (b n) -> p b n", b=B)
            o += B * N
            xit_s = blob_s[0:D, o : o + B * NI].rearrange(
                "p (b i) -> p b i", b=B
            )
            o += B * NI
            w1ad_s = blob_s[:, o : o + P]
            o += P
            w1c_s = blob_s[0:D, o : o + P]
            o += P
            w2bd_s = blob_s[:, o : o + P]
            o += P
            b1_s = blob_s[:, o : o + 1].bitcast(F32)
            o += 1
            assert o == BLOB_W

            # Setup per b: pa_stacked = blockdiag(W1a) @ [X^T; Y^T],
            # pc_stacked = [W1c_xx | W1c_yx]^T-style cat @ Xi^T, + b1.
            for b in range(B):
                ps = psum_pool.tile([P, G, 512], F32, tag="mm")
                nc.tensor.matmul(
                    ps[:, 0, 0:N],
                    lhsT=w1ad_s,
                    rhs=xyt_s[:, b, :],
                    start=True,
                    stop=True,
                )
                nc.vector.tensor_copy(pa_sb[:, b, :], ps[:, 0, 0:N])
                nc.tensor.matmul(
                    ps[:, 1, 0:NI],
                    lhsT=w1c_s,
                    rhs=xit_s[:, b, :],
                    start=True,
                    stop=True,
                )
                nc.vector.tensor_scalar_add(
                    out=pc_sb[:, b, :],
                    in0=ps[:, 1, 0:NI],
                    scalar1=b1_s,
                )

            # Main loop.
            flat = 0
            for b in range(B):
                for ig in range(NI // G):
                    ps = psum_pool.tile([P, G, 512], F32, tag="mm")
                    for g in range(G):
                        i = ig * G + g
                        rp = rp_pool.tile([P, N], F32R)
                        eng = PRE_PATTERN[flat % len(PRE_PATTERN)]
                        flat += 1
                        if eng == "act":
                            nc.scalar.activation(
                                out=rp[:, :],
                                in_=pa_sb[:, b, :],
                                func=ACTF.Relu,
                                bias=pc_sb[:, b, i : i + 1],
                                scale=1.0,
                            )
                        elif eng == "gpsimd":
                            nc.gpsimd.tensor_scalar(
                                out=rp[:, :],
                                in0=pa_sb[:, b, :],
                                scalar1=pc_sb[:, b, i : i + 1],
                                scalar2=0.0,
                                op0=ALU.add,
                                op1=ALU.max,
                            )
                        else:
                            nc.vector.tensor_scalar(
                                out=rp[:, :],
                                in0=pa_sb[:, b, :],
                                scalar1=pc_sb[:, b, i : i + 1],
                                scalar2=0.0,
                                op0=ALU.add,
                                op1=ALU.max,
                            )
                        nc.tensor.matmul(
                            ps[:, g, 0:N],
                            lhsT=w2bd_s,
                            rhs=rp[:, :],
                            start=True,
                            stop=True,
                        )
                    nc.vector.tensor_reduce(
                        out=strip[:, b, ig * G : ig * G + G],
                        in_=ps[:, :, 0:N],
                        axis=AX.X,
                        op=ALU.max,
                    )

            nc.vector.tensor_reduce(
                out=acc[:, :], in_=strip[:, :, :], axis=AX.X, op=ALU.add
            )
            nc.sync.dma_start(out=out[:, :], in_=acc[:, :])

    nc.compile()
    return nc


def _prep_inputs(X, Y, W_xx1, W_yx1, b_xx1, b_yx1, W_xx2, W_yx2):
    """Host-side input prep shared by all cores (except xit)."""
    f = np.float32
    XYT = np.ascontiguousarray(
        np.concatenate([X.transpose(0, 2, 1), Y.transpose(0, 2, 1)], axis=1), f
    )  # [B, 128, N]
    W1ad = np.zeros((P, P), f)
    W1ad[:D, :H] = W_xx1[:D]
    W1ad[D:, H:] = W_yx1[:D]
    W1c = np.ascontiguousarray(np.concatenate([W_xx1[D:], W_yx1[D:]], axis=1), f)
    b1v = np.concatenate([b_xx1, b_yx1]).reshape(P, 1).astype(f)
    W2bd = np.zeros((P, P), f)
    W2bd[:H, :H] = W_xx2
    W2bd[H:, H:] = W_yx2
    return XYT, W1ad, W1c, b1v, W2bd


def _pack_blob(XYT, XiT, W1ad, W1c, b1v, W2bd):
    """Pack all per-core inputs into the [P, BLOB_W] blob (see build_nc)."""
    f = np.float32
    blob = np.zeros((P, BLOB_W), f)
    o = 0
    blob[:, o : o + B * N] = XYT.transpose(1, 0, 2).reshape(P, B * N)
    o += B * N
    blob[:D, o : o + B * NI] = XiT.transpose(1, 0, 2).reshape(D, B * NI)
    o += B * NI
    blob[:, o : o + P] = W1ad
    o += P
    blob[:D, o : o + P] = W1c
    o += P
    blob[:, o : o + P] = W2bd
    o += P
    blob[:, o : o + 1] = b1v
    o += 1
    assert o == BLOB_W
    return blob


def kernel(
    X, Y,
    W_xx1, b_xx1, W_xx2, b_xx2,
    W_xy1, b_xy1, W_xy2, b_xy2,
    W_yx1, b_yx1, W_yx2, b_yx2,
    W_yy1, b_yy1, W_yy2, b_yy2,
    Wd1, bd1, Wd2, bd2,
    _trace=False, _tmpdir=None,
):
    f = np.float32
    X = np.asarray(X, f)
    Y = np.asarray(Y, f)
    XYT, W1ad, W1c, b1v, W2bd = _prep_inputs(
        X, Y, W_xx1, W_yx1, b_xx1, b_yx1, W_xx2, W_yx2
    )

    in_maps = []
    for c in range(NCORES):
        XiT = np.ascontiguousarray(
            X[:, c * NI : (c + 1) * NI, :].transpose(0, 2, 1), f
        )  # [B, 64, NI]
        in_maps.append({"blob": _pack_blob(XYT, XiT, W1ad, W1c, b1v, W2bd)})

    nc = build_nc()
    res = run_bass_kernel_spmd(
        nc,
        in_maps,
        core_ids=list(range(NCORES)),
        trace=_trace,
        tmpdir=_tmpdir,
    )
    acc = np.zeros((P, B), np.float64)
    for r in res.results:
        acc += r["out"].astype(np.float64)
    acc = acc.astype(f)

    # acc[k, b] = sum_i max_j (relu_pre @ W2)[k]  for xx (k<64) / yx (k>=64)
    Zdiff = (acc[:H] - acc[H:]).T + N * (b_xx2 - b_yx2)[None, :]  # [B, H]
    z = np.concatenate([Zdiff, Zdiff], axis=1).astype(f)  # [B, 2H]
    h = np.maximum(z @ Wd1 + bd1, 0.0).astype(f)
    outv = (h @ Wd2 + bd2).astype(f)
    if _trace:
        return outv, res
    return outv


# revision 11
# speedup vs baseline: 3.9420x; 1.0543x over previous
"""Trainium2 Bass kernel for nn_DivergenceRN (gnn_message_passing).

Reference computes, per batch b:
    Z_XX[b,i,:] = max_j relu(X[b,j]@W1a_xx + X[b,i]@W1c_xx + b1_xx) @ W_xx2
    Z_YX[b,i,:] = max_j relu(Y[b,j]@W1a_yx + X[b,i]@W1c_yx + b1_yx) @ W_yx2
    Z = sum_i (Z_XX - Z_YX);  out = relu(cat(Z,Z)@Wd1+bd1)@Wd2+bd2
(The YY / XY branches in the reference are dead code — output-independent.)

Device layout: partitions = 64 h-channels x {xx, yx} = 128; free dim = j.
Per (b,i): relu_pre = relu(pa_stacked + pc_col)  (ACT / GPSIMD / DVE),
one float32r matmul vs blockdiag(W_xx2, W_yx2)  (PE), max over j (DVE
reduce, batched over 4 PSUM banks), sum over i into a strip buffer.
Sharding: i in [0,384) split across 8 cores (48 rows per core per batch).
Host does the tiny decoder + bias folding.
"""

import numpy as np

import concourse.bacc as bacc
import concourse.bass as bass
import concourse.mybir as mybir
import concourse.tile as tile
from concourse.bass_utils import run_bass_kernel_spmd

B, N, M, D, H = 4, 384, 384, 64, 64
NCORES = 8
NI = N // NCORES          # i-rows per core per batch
G = 4                     # i's reduced per DVE tensor_reduce (PSUM banks)
P = 2 * H                 # 128 partitions: h x {xx, yx}
BLOB_W = B * N + B * NI + 2 * P + 1   # packed input blob columns

F32 = mybir.dt.float32
F32R = mybir.dt.float32r
BF16 = mybir.dt.bfloat16
AX = mybir.AxisListType
ALU = mybir.AluOpType
ACTF = mybir.ActivationFunctionType

# relu_pre engine assignment, cycled over the flat (b, i) index.
PRE_PATTERN = ("act", "act", "act", "act", "dve")


def build_nc():
    nc = bacc.Bacc("TRN2", target_bir_lowering=False)

    # All inputs packed into one [128, BLOB_W] f32 blob: a single DMA =
    # a single semaphore (per-instruction sync-wait limits are tiny).
    blob = nc.dram_tensor("blob", [P, BLOB_W], F32R, kind="ExternalInput")
    w2bd16 = nc.dram_tensor("w2bd16", [P, P], BF16, kind="ExternalInput")
    out = nc.dram_tensor("out", [P, B], F32, kind="ExternalOutput")

    with tile.TileContext(nc) as tc:
        with (
            tc.tile_pool(name="singles", bufs=1) as singles,
            tc.tile_pool(name="rp", bufs=16) as rp_pool,
            tc.tile_pool(name="psum", bufs=2, space="PSUM") as psum_pool,
        ):
            blob_s = singles.tile([P, BLOB_W], F32R)
            w2bd_s16 = singles.tile([P, P], BF16)
            pa_sb = singles.tile([P, B, N], F32)
            pc_sb = singles.tile([P, B, NI], F32)
            strip = singles.tile([P, B, NI], F32)
            acc = singles.tile([P, B], F32)

            nc.sync.dma_start(out=blob_s[:, :], in_=blob[:, :])
            o = 0
            xyt_s = blob_s[:, o : o + B * N].rearrange("p (b n) -> p b n", b=B)
            o += B * N
            xit_s = blob_s[0:D, o : o + B * NI].rearrange(
                "p (b i) -> p b i", b=B
            )
            o += B * NI
            w1ad_s = blob_s[:, o : o + P]
            o += P
            w1c_s = blob_s[0:D, o : o + P]
            o += P
            nc.sync.dma_start(out=w2bd_s16, in_=w2bd16[:, :])
            b1_s = blob_s[:, o : o + 1].bitcast(F32)
            o += 1
            assert o == BLOB_W

            # Setup per b: pa_stacked = blockdiag(W1a) @ [X^T; Y^T],
            # pc_stacked = [W1c_xx | W1c_yx]^T-style cat @ Xi^T, + b1.
            for b in range(B):
                ps = psum_pool.tile([P, G, 512], F32, tag="mm")
                nc.tensor.matmul(
                    ps[:, 0, 0:N],
                    lhsT=w1ad_s,
                    rhs=xyt_s[:, b, :],
                    start=True,
                    stop=True,
                )
                nc.vector.tensor_copy(pa_sb[:, b, :], ps[:, 0, 0:N])
                nc.tensor.matmul(
                    ps[:, 1, 0:NI],
                    lhsT=w1c_s,
                    rhs=xit_s[:, b, :],
                    start=True,
                    stop=True,
                )
                nc.vector.tensor_scalar_add(
                    out=pc_sb[:, b, :],
                    in0=ps[:, 1, 0:NI],
                    scalar1=b1_s,
                )

            # Main loop.
            flat = 0
            for b in range(B):
                for ig in range(NI // G):
                    ps = psum_pool.tile([P, G, 512], F32, tag="mm")
                    for g in range(G):
                        i = ig * G + g
                        rp = rp_pool.tile([P, N], BF16)
                        eng = PRE_PATTERN[flat % len(PRE_PATTERN)]
                        flat += 1
                        if eng == "act":
                            nc.scalar.activation(
                                out=rp[:, :],
                                in_=pa_sb[:, b, :],
                                func=ACTF.Relu,
                                bias=pc_sb[:, b, i : i + 1],
                                scale=1.0,
                            )
                        elif eng == "gpsimd":
                            nc.gpsimd.tensor_scalar(
                                out=rp[:, :],
                                in0=pa_sb[:, b, :],
                                scalar1=pc_sb[:, b, i : i + 1],
                                scalar2=0.0,
                                op0=ALU.add,
                                op1=ALU.max,
                            )
                        else:
                            nc.vector.tensor_scalar(
                                out=rp[:, :],
                                in0=pa_sb[:, b, :],
                                scalar1=pc_sb[:, b, i : i + 1],
                                scalar2=0.0,
                                op0=ALU.add,
                                op1=ALU.max,
                            )
                        nc.tensor.matmul(
                            ps[:, g, 0:N],
                            lhsT=w2bd_s16[:, :],
                            rhs=rp[:, :],
                            start=True,
                            stop=True,
                        )
                    nc.vector.tensor_reduce(
                        out=strip[:, b, ig * G : ig * G + G],
                        in_=ps[:, :, 0:N],
                        axis=AX.X,
                        op=ALU.max,
                    )

            nc.vector.tensor_reduce(
                out=acc[:, :], in_=strip[:, :, :], axis=AX.X, op=ALU.add
            )
            nc.sync.dma_start(out=out[:, :], in_=acc[:, :])

    nc.compile()
    return nc


def _prep_inputs(X, Y, W_xx1, W_yx1, b_xx1, b_yx1, W_xx2, W_yx2):
    """Host-side input prep shared by all cores (except xit)."""
    f = np.float32
    XYT = np.ascontiguousarray(
        np.concatenate([X.transpose(0, 2, 1), Y.transpose(0, 2, 1)], axis=1), f
    )  # [B, 128, N]
    W1ad = np.zeros((P, P), f)
    W1ad[:D, :H] = W_xx1[:D]
    W1ad[D:, H:] = W_yx1[:D]
    W1c = np.ascontiguousarray(np.concatenate([W_xx1[D:], W_yx1[D:]], axis=1), f)
    b1v = np.concatenate([b_xx1, b_yx1]).reshape(P, 1).astype(f)
    W2bd = np.zeros((P, P), f)
    W2bd[:H, :H] = W_xx2
    W2bd[H:, H:] = W_yx2
    return XYT, W1ad, W1c, b1v, W2bd


def _pack_blob(XYT, XiT, W1ad, W1c, b1v, W2bd):
    """Pack all per-core inputs into the [P, BLOB_W] blob (see build_nc)."""
    f = np.float32
    blob = np.zeros((P, BLOB_W), f)
    o = 0
    blob[:, o : o + B * N] = XYT.transpose(1, 0, 2).reshape(P, B * N)
    o += B * N
    blob[:D, o : o + B * NI] = XiT.transpose(1, 0, 2).reshape(D, B * NI)
    o += B * NI
    blob[:, o : o + P] = W1ad
    o += P
    blob[:D, o : o + P] = W1c
    o += P
    blob[:, o : o + 1] = b1v
    o += 1
    assert o == BLOB_W
    return blob


def kernel(
    X, Y,
    W_xx1, b_xx1, W_xx2, b_xx2,
    W_xy1, b_xy1, W_xy2, b_xy2,
    W_yx1, b_yx1, W_yx2, b_yx2,
    W_yy1, b_yy1, W_yy2, b_yy2,
    Wd1, bd1, Wd2, bd2,
    _trace=False, _tmpdir=None,
):
    f = np.float32
    X = np.asarray(X, f)
    Y = np.asarray(Y, f)
    XYT, W1ad, W1c, b1v, W2bd = _prep_inputs(
        X, Y, W_xx1, W_yx1, b_xx1, b_yx1, W_xx2, W_yx2
    )
    import ml_dtypes
    W2bd16 = np.ascontiguousarray(W2bd.astype(ml_dtypes.bfloat16))

    in_maps = []
    for c in range(NCORES):
        XiT = np.ascontiguousarray(
            X[:, c * NI : (c + 1) * NI, :].transpose(0, 2, 1), f
        )  # [B, 64, NI]
        in_maps.append(
            {
                "blob": _pack_blob(XYT, XiT, W1ad, W1c, b1v, W2bd),
                "w2bd16": W2bd16,
            }
        )

    nc = build_nc()
    res = run_bass_kernel_spmd(
        nc,
        in_maps,
        core_ids=list(range(NCORES)),
        trace=_trace,
        tmpdir=_tmpdir,
    )
    acc = np.zeros((P, B), np.float64)
    for r in res.results:
        acc += r["out"].astype(np.float64)
    acc = acc.astype(f)

    # acc[k, b] = sum_i max_j (relu_pre @ W2)[k]  for xx (k<64) / yx (k>=64)
    Zdiff = (acc[:H] - acc[H:]).T + N * (b_xx2 - b_yx2)[None, :]  # [B, H]
    z = np.concatenate([Zdiff, Zdiff], axis=1).astype(f)  # [B, 2H]
    h = np.maximum(z @ Wd1 + bd1, 0.0).astype(f)
    outv = (h @ Wd2 + bd2).astype(f)
    if _trace:
        return outv, res
    return outv
